# revision 1
# baseline (speedup 1.0000x reference)
"""Mass-spring substep integrator on 8 Trainium2 NeuronCores.

Topology: all 8 cores carry the full node state for ALL 4 batch elements;
the 400k springs are split into 8 per-node-balanced shards, one per core.
Each substep every core computes the partial per-node forces of its shard
for all 4 batches at once, the partials are AllReduced across the 8 cores,
and every core integrates the full state identically.

Per-core data layout ("owner grid"):
  - nodes are relabeled on the host: sorted by incidence count and dealt
    round-robin onto the 128 SBUF partitions -> node (p, k).
  - the directed incidences (edge endpoints) of a core's shard are laid out
    in a [128, J] slot grid grouped by owner node, with a degree-profile
    template (segment sizes per rank k) SHARED across partitions and shards,
    so owner-side broadcast / segmented reduction are plain strided
    (lockstep) vector ops.
  - the partner position of every slot (all 4 batches x 3 comps = one
    48-byte record) is fetched with indirect DMA from a DRAM copy of the
    positions, one gather column (128 slots) per call.

All node/slot index tables are precomputed on the host from the (static)
edge list; outputs are un-permuted back to the original node order on the
host after the device run.
"""

import numpy as np

import concourse.bass as bass
import concourse.mybir as mybir
import concourse.tile as tile
from concourse.bass_utils import run_bass_kernel_spmd

# Problem constants (must match the reference)
B, NV, NE, SUBSTEPS = 4, 100000, 400000, 10
DT = 0.01
K_SPRING = 1000.0
MASS = 1.0
DAMP = 0.999
ACT_SCALE = 0.1
EPS = 1e-6
GRAVITY_Y = -9.8

P = 128           # SBUF partitions
NSHARD = 8        # edge shards == cores
PAD_REST = float(np.sqrt(EPS))  # rest length that zeroes force on d=0 pad slots


# ---------------------------------------------------------------------------
# walrus workaround: this toolchain accepts only ONE sync-wait per
# instruction; split extra waits onto fresh same-engine NOPs.
# ---------------------------------------------------------------------------
_ctr = [0]


def _split_multi_waits(nc):
    for f in nc.m.functions:
        for b in f.blocks:
            old = b.instructions
            new = []
            changed = False
            for inst in old:
                si = inst.sync_info
                if si is not None and si.on_wait is not None and len(si.on_wait) > 1:
                    waits = list(si.on_wait)
                    for w in waits[:-1]:
                        _ctr[0] += 1
                        nop = mybir.InstNoOp(
                            name=f"SPLITW-{_ctr[0]}",
                            engine=inst.engine,
                            ins=[], outs=[],
                            sync_info=mybir.SyncInfo(on_wait=[w], on_update=[]),
                        )
                        new.append(nop)
                    si.on_wait = waits[-1:]
                    changed = True
                new.append(inst)
            if changed:
                b.instructions = new


class _TileContext(tile.TileContext):
    def __exit__(self, *args):
        r = super().__exit__(*args)
        if args[0] is None:
            _split_multi_waits(self.nc)
        return r


# ---------------------------------------------------------------------------
# Host-side plan construction (static, depends only on the edge list)
# ---------------------------------------------------------------------------
class Plan:
    pass


def build_plan(edges, nv, ne):
    """Relabel nodes, split edges into NSHARD balanced shards, build the
    shared degree-profile slot template and per-shard index tables."""
    rng = np.random.RandomState(0)
    nvp = -(-nv // P)            # nodes per partition (ceil)
    nvtot = nvp * P

    i_idx = edges[:, 0].astype(np.int64)
    j_idx = edges[:, 1].astype(np.int64)

    # --- balanced split of edges into NSHARD shards (per-node incidence) ---
    order = rng.permutation(ne)
    cnt = np.zeros((NSHARD, nv), np.int32)
    shard_of_edge = np.zeros(ne, np.int8)
    ii, jj = i_idx[order], j_idx[order]
    for t in range(ne):
        u = ii[t]
        v = jj[t]
        s = int(np.argmin(cnt[:, u] + cnt[:, v]))
        shard_of_edge[order[t]] = s
        cnt[s, u] += 1
        cnt[s, v] += 1

    deg_h = cnt  # [NSHARD, NV]

    # --- node ranking: sort by max shard-degree desc, deal round-robin ---
    key = deg_h.max(axis=0)
    node_order = np.argsort(-key, kind="stable")
    node_order_pad = np.concatenate([node_order, np.arange(nv, nvtot)])
    grid_nodes = node_order_pad.reshape(nvp, P)  # [k, p]
    p_of = np.zeros(nvtot, np.int32)
    k_of = np.zeros(nvtot, np.int32)
    p_of[grid_nodes.ravel()] = np.tile(np.arange(P), nvp)
    k_of[grid_nodes.ravel()] = np.repeat(np.arange(nvp), P)

    # --- shared degree template: D[k] = max over shards & partitions ---
    degh_pad = np.zeros((NSHARD, nvtot), np.int32)
    degh_pad[:, :nv] = deg_h
    dk = np.max(degh_pad[:, grid_nodes], axis=(0, 2))  # [nvp]
    rk_order = np.argsort(-dk, kind="stable")
    grid_nodes = grid_nodes[rk_order]
    dk = dk[rk_order]
    k_of[grid_nodes.ravel()] = np.repeat(np.arange(nvp), P)

    seg_start = np.zeros(nvp + 1, np.int64)
    seg_start[1:] = np.cumsum(dk)
    J = int(seg_start[-1])

    # degree classes: runs of equal dk with dk >= 1
    classes = []
    k = 0
    while k < nvp:
        d = int(dk[k])
        k2 = k
        while k2 < nvp and dk[k2] == d:
            k2 += 1
        if d >= 1:
            classes.append((k, k2, d))
        k = k2

    # split classes into chunks of bounded slot count (for SBUF);
    # classes may split at rank boundaries.
    nchunk = 3
    target = -(-J // nchunk)
    cls_chunks = [[]]
    cur = 0
    for (ka, kb, d) in classes:
        k0 = ka
        while k0 < kb:
            room = max((target - cur) // d, 0)
            take = min(kb - k0, room)
            if take == 0:
                cls_chunks.append([])
                cur = 0
                continue
            cls_chunks[-1].append((k0, k0 + take, d))
            cur += take * d
            k0 += take
    cls_chunks = [ch for ch in cls_chunks if ch]
    chunk_bounds = [
        (int(seg_start[ch[0][0]]), int(seg_start[ch[-1][1]]))
        for ch in cls_chunks
    ]

    # --- per-shard slot tables ---
    flat_of = (p_of.astype(np.int64) * nvp + k_of)
    part_idx = np.zeros((NSHARD, P, J), np.int32)
    eidx_slot = np.full((NSHARD, P, J), -1, np.int64)

    owner_flat = np.zeros((P, J), np.int64)
    for (ka, kb, d) in classes:
        for krank in range(ka, kb):
            s0 = seg_start[krank]
            owner_flat[:, s0:s0 + d] = (
                np.arange(P, dtype=np.int64)[:, None] * nvp + krank
            )

    for h in range(NSHARD):
        sel = shard_of_edge == h
        eu = np.concatenate([i_idx[sel], j_idx[sel]])
        ev = np.concatenate([j_idx[sel], i_idx[sel]])
        ee = np.concatenate([np.nonzero(sel)[0]] * 2)
        owner_p = p_of[eu]
        owner_k = k_of[eu]
        so = np.lexsort((ee, owner_k, owner_p))
        eu, ev, ee = eu[so], ev[so], ee[so]
        owner_p, owner_k = owner_p[so], owner_k[so]
        grp = owner_p.astype(np.int64) * nvp + owner_k
        uniq, first = np.unique(grp, return_index=True)
        within = np.arange(len(grp)) - np.repeat(
            first, np.diff(np.append(first, len(grp))))
        jpos = seg_start[owner_k] + within
        part_idx[h, owner_p, jpos] = flat_of[ev]
        eidx_slot[h, owner_p, jpos] = ee
        padmask = eidx_slot[h] < 0
        part_idx[h][padmask] = owner_flat[padmask].astype(np.int32)

    plan = Plan()
    plan.nv, plan.ne, plan.nvp, plan.nvtot, plan.J = nv, ne, nvp, nvtot, J
    plan.classes = classes
    plan.cls_chunks = cls_chunks
    plan.chunk_bounds = chunk_bounds
    plan.seg_start = seg_start
    plan.part_idx = part_idx
    plan.eidx_slot = eidx_slot
    plan.p_of, plan.k_of = p_of, k_of
    plan.grid_nodes = grid_nodes
    return plan


def host_state_inputs(plan, input_pos, input_vel):
    """Shared (all-core) initial state in internal layout [P, M*nvp],
    plane m = b*3 + c."""
    nvp = plan.nvp
    nv = plan.nv
    nb = input_pos.shape[0]
    gn = plan.grid_nodes  # [k, p]
    valid = gn < nv
    gp = np.clip(gn, 0, nv - 1)
    ps = input_pos[:, gp].copy()   # [b, k, p, 3]
    vs = input_vel[:, gp].copy()
    ps[:, ~valid] = 0.0
    vs[:, ~valid] = 0.0
    pos = ps.transpose(2, 0, 3, 1).reshape(P, nb * 3 * nvp)
    vel = vs.transpose(2, 0, 3, 1).reshape(P, nb * 3 * nvp)
    return (np.ascontiguousarray(pos, dtype=np.float32),
            np.ascontiguousarray(vel, dtype=np.float32))


def host_shard_inputs(plan, h, input_action, rest_len):
    """Per-core shard tables: pidx [P,J] i32, rest [P,J] f32,
    act [P, J*NB] f32 (b innermost)."""
    J = plan.J
    nb = input_action.shape[0]
    e = plan.eidx_slot[h]
    pad = e < 0
    ec = np.clip(e, 0, plan.ne - 1)
    rest = rest_len[ec].astype(np.float32)
    rest[pad] = PAD_REST
    act = input_action[:, ec].astype(np.float32)  # [b, P, J]
    act[:, pad] = 0.0
    act = np.ascontiguousarray(act.transpose(1, 2, 0).reshape(P, J * nb))
    return {
        "pidx": np.ascontiguousarray(plan.part_idx[h]),
        "rest_s": np.ascontiguousarray(rest),
        "act_s": act,
    }


def unpermute_output(plan, traj, nb):
    """traj [S+1, P, nb*3, nvp] internal -> [nb, S+1, NV, 3]."""
    pv = plan.p_of[: plan.nv]
    kv = plan.k_of[: plan.nv]
    t = traj.reshape(traj.shape[0], P, nb, 3, plan.nvp)
    out = t[:, pv, :, :, kv]        # [NV, S+1, nb, 3]
    return np.ascontiguousarray(out.transpose(2, 1, 0, 3))


# ---------------------------------------------------------------------------
# Device kernel
# ---------------------------------------------------------------------------
def build_bass(plan, substeps, nb):
    nvp, J, nvtot = plan.nvp, plan.J, plan.nvtot
    m = nb * 3
    NPM = m * nvp
    f32 = mybir.dt.float32

    nc = bass.Bass(num_devices=8)
    pos0 = nc.dram_tensor("pos0", [P, NPM], f32, kind="ExternalInput")
    vel0 = nc.dram_tensor("vel0", [P, NPM], f32, kind="ExternalInput")
    pidx = nc.dram_tensor("pidx", [P, J], mybir.dt.int32, kind="ExternalInput")
    rest_in = nc.dram_tensor("rest_s", [P, J], f32, kind="ExternalInput")
    act_in = nc.dram_tensor("act_s", [P, J * nb], f32, kind="ExternalInput")

    opos = nc.dram_tensor("opos", [substeps + 1, P, NPM], f32,
                          kind="ExternalOutput")
    ovel = nc.dram_tensor("ovel", [substeps + 1, P, NPM], f32,
                          kind="ExternalOutput")

    ptab = nc.dram_tensor("ptab", [nvtot, m], f32, kind="Internal")
    cc_in = nc.dram_tensor("cc_in", [P, NPM], f32, kind="Internal")
    cc_out = nc.dram_tensor("cc_out", [P, NPM], f32, kind="Internal")

    chunks = plan.chunk_bounds
    maxchunk = max(hi - lo for (lo, hi) in chunks)

    with _TileContext(nc) as tc:
        with tc.tile_pool(name="state", bufs=1) as pool:
            pos = pool.tile([P, NPM], f32, name="pos")
            vel = pool.tile([P, NPM], f32, name="vel")
            fsum = pool.tile([P, NPM], f32, name="fsum")
            pidx_sb = pool.tile([P, J], mybir.dt.int32, name="pidx_sb")
            kr = pool.tile([P, J * nb], f32, name="kr")
            s2 = pool.tile([P, J * nb], f32, name="s2")
            sq = pool.tile([P, maxchunk * nb], f32, name="sq")
            rem = pool.tile([P, maxchunk * m], f32, name="rem")
            eps_t = pool.tile([P, 1], f32, name="eps_t")

            pos_mk = pos[:].rearrange("p (m k) -> p m k", m=m)
            fsum_mk = fsum[:].rearrange("p (m k) -> p m k", m=m)

            def _ins_bcast(ap, pos_idx, count):
                dims = [list(x) for x in ap.ap]
                dims.insert(pos_idx, [0, count])
                return bass.AP(ap.tensor, ap.offset, dims)

            # ---- one-time setup ----
            nc.vector.memset(eps_t[:], float(EPS))
            nc.sync.dma_start(pos[:], pos0[:])
            nc.sync.dma_start(vel[:], vel0[:])
            nc.sync.dma_start(pidx_sb[:], pidx[:])
            # kr[p, j, b] = K * rest[p, j] * (1 + ACT_SCALE * tanh(act))
            act_t = s2[:]
            nc.sync.dma_start(act_t, act_in[:])
            nc.scalar.activation(kr[:], act_t,
                                 mybir.ActivationFunctionType.Tanh)
            nc.vector.tensor_scalar(
                out=kr[:], in0=kr[:], scalar1=float(ACT_SCALE),
                scalar2=float(1.0), op0=mybir.AluOpType.mult,
                op1=mybir.AluOpType.add)
            rest_t = rem[:, 0:J]
            nc.sync.dma_start(rest_t, rest_in[:])
            kr_v = kr[:].rearrange("p (j b) -> p j b", b=nb)
            rest_b = _ins_bcast(rest_t, 2, nb)
            nc.vector.tensor_tensor(out=kr_v, in0=kr_v, in1=rest_b,
                                    op=mybir.AluOpType.mult)
            nc.vector.tensor_scalar_mul(kr[:], kr[:], float(K_SPRING))

            # initial state into trajectory
            nc.sync.dma_start(opos[0], pos[:])
            nc.sync.dma_start(ovel[0], vel[:])

            # ---- substeps (statically unrolled) ----
            for s in range(substeps):
                # 1) node positions -> DRAM table [nvtot, m]
                for mm in range(m):
                    for ph in (0, 1):
                        pr = ptab[ph * 64 * nvp:(ph + 1) * 64 * nvp,
                                  mm:mm + 1]
                        nc.sync.dma_start(
                            pr.rearrange("(p k) o -> p k o", p=64),
                            pos[ph * 64:(ph + 1) * 64,
                                mm * nvp:(mm + 1) * nvp],
                        )
                nc.vector.memset(fsum[:], 0.0)

                for ci, (lo, hi) in enumerate(chunks):
                    cw = hi - lo
                    rem_v = rem[:, :cw * m].rearrange(
                        "p (j r) -> p j r", r=m)          # [P, cw, m]
                    # 2) bridge: one gather column per slot
                    for j in range(lo, hi):
                        nc.gpsimd.indirect_dma_start(
                            out=rem[:, (j - lo) * m:(j - lo + 1) * m],
                            out_offset=None,
                            in_=ptab[:],
                            in_offset=bass.IndirectOffsetOnAxis(
                                ap=pidx_sb[:, j:j + 1], axis=0),
                        )
                    # 3) d = rem - own (per degree class)
                    for (ka, kb, d) in plan.cls_chunks[ci]:
                        s0 = int(plan.seg_start[ka]) - lo
                        nk = kb - ka
                        dst = rem_v[:, s0:s0 + nk * d, :].rearrange(
                            "p (n dd) r -> p n dd r", dd=d)
                        src = pos_mk[:, :, ka:kb].rearrange("p m n -> p n m")
                        src = _ins_bcast(src, 2, d)
                        nc.vector.tensor_tensor(
                            out=dst, in0=dst, in1=src,
                            op=mybir.AluOpType.subtract)
                    # 4) s2[j, b] = sum_c d_c^2
                    s2c = s2[:, lo * nb:hi * nb]
                    s2v = s2c.rearrange("p (j b) -> p j b", b=nb)
                    sqc = sq[:, :cw * nb]
                    sqv = sqc.rearrange("p (j b) -> p j b", b=nb)
                    rem_jbc = rem[:, :cw * m].rearrange(
                        "p (j b c) -> p j b c", b=nb, c=3)
                    cviews = [rem_jbc[:, :, :, c] for c in range(3)]
                    nc.vector.tensor_tensor(out=s2v, in0=cviews[0],
                                            in1=cviews[0],
                                            op=mybir.AluOpType.mult)
                    for c in (1, 2):
                        nc.vector.tensor_tensor(out=sqv, in0=cviews[c],
                                                in1=cviews[c],
                                                op=mybir.AluOpType.mult)
                        nc.vector.tensor_tensor(out=s2v, in0=s2v, in1=sqv,
                                                op=mybir.AluOpType.add)
                    # length = sqrt(s2+eps); inv = 1/length (into sq)
                    nc.scalar.activation(
                        s2c, s2c, mybir.ActivationFunctionType.Sqrt,
                        bias=eps_t[:])
                    nc.vector.reciprocal(sqc, s2c)
                    # coef = K - kr/len   (into s2)
                    nc.vector.tensor_tensor(
                        out=s2c, in0=sqc, in1=kr[:, lo * nb:hi * nb],
                        op=mybir.AluOpType.mult)
                    nc.scalar.activation(
                        s2c, s2c, mybir.ActivationFunctionType.Copy,
                        bias=float(K_SPRING), scale=-1.0)
                    # 5) f = coef * d (in place)
                    coef_b = _ins_bcast(s2v, 3, 3)
                    nc.vector.tensor_tensor(
                        out=rem_jbc, in0=rem_jbc, in1=coef_b,
                        op=mybir.AluOpType.mult)
                    # 6) segmented reduce -> fsum planes
                    for (ka, kb, d) in plan.cls_chunks[ci]:
                        s0 = int(plan.seg_start[ka]) - lo
                        nk = kb - ka
                        src = rem_v[:, s0:s0 + nk * d, :].rearrange(
                            "p (n dd) r -> p n r dd", dd=d)
                        dst = fsum_mk[:, :, ka:kb].rearrange("p m n -> p n m")
                        nc.vector.tensor_reduce(
                            out=dst, in_=src, axis=mybir.AxisListType.X,
                            op=mybir.AluOpType.add)

                # 7) AllReduce partial forces across the 8 shards
                nc.sync.dma_start(cc_in[:], fsum[:])
                nc.gpsimd.collective_compute(
                    "AllReduce", mybir.AluOpType.add,
                    replica_groups=[list(range(8))],
                    ins=[cc_in[:]], outs=[cc_out[:]],
                )
                nc.sync.dma_start(fsum[:], cc_out[:])
                # 8) integrate:
                #    fsum = fsum*DT + vel ; fsum_y += DT*G (per batch)
                #    vel = fsum*DAMP ; pos = vel*DT + pos
                nc.vector.scalar_tensor_tensor(
                    out=fsum[:], in0=fsum[:], scalar=float(DT / MASS),
                    in1=vel[:], op0=mybir.AluOpType.mult,
                    op1=mybir.AluOpType.add)
                for b in range(nb):
                    mm = b * 3 + 1
                    nc.vector.tensor_scalar_add(
                        fsum[:, mm * nvp:(mm + 1) * nvp],
                        fsum[:, mm * nvp:(mm + 1) * nvp],
                        float(GRAVITY_Y * DT))
                nc.vector.tensor_scalar_mul(vel[:], fsum[:], float(DAMP))
                nc.vector.scalar_tensor_tensor(
                    out=pos[:], in0=vel[:], scalar=float(DT),
                    in1=pos[:], op0=mybir.AluOpType.mult,
                    op1=mybir.AluOpType.add)
                # 9) write trajectory
                nc.sync.dma_start(opos[s + 1], pos[:])
                nc.sync.dma_start(ovel[s + 1], vel[:])

    return nc


# ---------------------------------------------------------------------------
# Entry point
# ---------------------------------------------------------------------------
_cache = {}


def _get_plan_and_bass(edges, nv, ne, substeps, nb):
    kh = (hash(edges.tobytes()), nv, ne, substeps, nb)
    if kh not in _cache:
        plan = build_plan(edges, nv, ne)
        nc = build_bass(plan, substeps, nb)
        _cache[kh] = (plan, nc)
    return _cache[kh]


def kernel(input_action, input_pos, input_vel, rest_len, edges):
    input_action = np.asarray(input_action, np.float32)
    input_pos = np.asarray(input_pos, np.float32)
    input_vel = np.asarray(input_vel, np.float32)
    rest_len = np.asarray(rest_len, np.float32)
    edges = np.asarray(edges, np.int32)

    nb, nv, _ = input_pos.shape
    ne = edges.shape[0]
    plan, nc = _get_plan_and_bass(edges, nv, ne, SUBSTEPS, nb)

    pos0, vel0 = host_state_inputs(plan, input_pos, input_vel)
    in_maps = []
    for c in range(8):
        im = {"pos0": pos0, "vel0": vel0}
        im.update(host_shard_inputs(plan, c, input_action, rest_len))
        in_maps.append(im)
    res = run_bass_kernel_spmd(nc, in_maps, core_ids=list(range(8)))

    r = res.results[0]
    tp = r["opos"].reshape(SUBSTEPS + 1, P, nb * 3, plan.nvp)
    tv = r["ovel"].reshape(SUBSTEPS + 1, P, nb * 3, plan.nvp)
    out_pos = unpermute_output(plan, tp, nb)
    out_vel = unpermute_output(plan, tv, nb)
    return out_pos, out_vel



# revision 7
# speedup vs baseline: 2.2831x; 2.2831x over previous
"""Mass-spring substep integrator on 8 Trainium2 NeuronCores.

Topology (node-sliced, v2):
  - Nodes are sorted by incidence count and grouped into KSL=98 rank-blocks
    of 1024; each block is dealt across the 8 cores x 128 partitions, so
    core c owns nodes at (c, p, t) for t in [0, 98).  Each core processes
    ALL directed incidences whose owner node lies in its slice, so per-node
    force sums are core-local (no force AllReduce).
  - The per-rank slot template D[t] = max incidence count over the 1024
    nodes of block t is shared across cores and partitions, so owner-side
    broadcast / segmented reduction are plain strided vector ops.
  - Each substep ends with one AllGather of the (negated, fp16) positions
    into a node-record table [100352, 12] that feeds the next substep's
    partner gather: a few large multi-offset indirect DMAs whose CCE add
    against an owner-position prefill materializes -d directly in SBUF.
  - Integration runs in fp32 on the owned slice only; each core writes its
    slice of the trajectory and the host stitches + unpermutes.
"""

import numpy as np

import concourse.bass as bass
import concourse.mybir as mybir
import concourse.tile as tile
from concourse.bass_utils import run_bass_kernel_spmd

# Problem constants (must match the reference)
B, NV, NE, SUBSTEPS = 4, 100000, 400000, 10
DT = 0.01
K_SPRING = 1000.0
MASS = 1.0
DAMP = 0.999
ACT_SCALE = 0.1
EPS = 1e-6
GRAVITY_Y = -9.8

P = 128            # SBUF partitions
NCORE = 8
KSL = 98           # node ranks per core
NBLK = P * NCORE   # nodes per rank-block (across all cores)
NVTOT = KSL * NBLK # padded node count (100352)
M = B * 3          # per-node record: 4 batches x 3 comps
NPM = KSL * M      # per-partition state floats (layout: t outer, m inner)
NCHUNK = 3         # gather pipeline chunks


# ---------------------------------------------------------------------------
# walrus workaround: this toolchain accepts only ONE sync-wait per
# instruction; split extra waits onto fresh same-engine NOPs.
# ---------------------------------------------------------------------------
_ctr = [0]


def _split_multi_waits(nc):
    for f in nc.m.functions:
        for b in f.blocks:
            old = b.instructions
            new = []
            changed = False
            for inst in old:
                si = inst.sync_info
                if si is not None and si.on_wait is not None and len(si.on_wait) > 1:
                    waits = list(si.on_wait)
                    for w in waits[:-1]:
                        _ctr[0] += 1
                        nop = mybir.InstNoOp(
                            name=f"SPLITW-{_ctr[0]}",
                            engine=inst.engine,
                            ins=[], outs=[],
                            sync_info=mybir.SyncInfo(on_wait=[w], on_update=[]),
                        )
                        new.append(nop)
                    si.on_wait = waits[-1:]
                    changed = True
                new.append(inst)
            if changed:
                b.instructions = new


class _TileContext(tile.TileContext):
    def __exit__(self, *args):
        r = super().__exit__(*args)
        if args[0] is None:
            _split_multi_waits(self.nc)
        return r


# ---------------------------------------------------------------------------
# Host-side plan construction (static, depends only on the edge list)
# ---------------------------------------------------------------------------
class Plan:
    pass


def build_plan(edges, nv, ne):
    u = np.concatenate([edges[:, 0], edges[:, 1]]).astype(np.int64)
    v = np.concatenate([edges[:, 1], edges[:, 0]]).astype(np.int64)
    eid = np.concatenate([np.arange(ne)] * 2)

    deg = np.bincount(u, minlength=nv)
    order = np.argsort(-deg, kind="stable")
    sorted_pad = np.concatenate([order, np.arange(nv, NVTOT)])
    blocks = sorted_pad.reshape(KSL, NBLK)          # [t, j]

    jj = np.arange(NBLK)
    t_of = np.zeros(NVTOT, np.int64)
    c_of = np.zeros(NVTOT, np.int64)
    p_of = np.zeros(NVTOT, np.int64)
    for t in range(KSL):
        nodes = blocks[t]
        t_of[nodes] = t
        c_of[nodes] = (jj + t) % NCORE
        p_of[nodes] = jj // NCORE
    row_of = c_of * (KSL * P) + t_of * P + p_of

    degpad = np.zeros(NVTOT, np.int64)
    degpad[:nv] = deg
    D = degpad[blocks].max(axis=1)                  # [KSL]
    seg = np.zeros(KSL + 1, np.int64)
    seg[1:] = np.cumsum(D)
    J = int(seg[-1])

    classes = []
    t0 = 0
    while t0 < KSL:
        t1 = t0
        while t1 < KSL and D[t1] == D[t0]:
            t1 += 1
        if D[t0] >= 1:
            classes.append((t0, t1, int(D[t0])))
        t0 = t1

    # per-core slot tables: default partner = self (pad slots -> d = 0)
    pidx = np.zeros((NCORE, P, J), np.int32)
    self_rows = np.zeros((NCORE, P, KSL), np.int64)
    self_rows[c_of, p_of, t_of] = row_of
    for (ta, tb, d) in classes:
        for t in range(ta, tb):
            pidx[:, :, seg[t]:seg[t] + d] = self_rows[:, :, t, None]
    eslot = np.full((NCORE, P, J), -1, np.int64)    # edge id per slot

    so = np.lexsort((eid, u))
    us, vs, es = u[so], v[so], eid[so]
    first = np.searchsorted(us, np.arange(nv))
    cnt = np.arange(len(us)) - first[us]
    slot = seg[t_of[us]] + cnt
    pidx[c_of[us], p_of[us], slot] = row_of[vs].astype(np.int32)
    eslot[c_of[us], p_of[us], slot] = es

    plan = Plan()
    plan.nv, plan.ne, plan.J = nv, ne, J
    plan.classes = classes
    plan.seg = seg
    plan.pidx = pidx
    plan.eslot = eslot
    plan.c_of, plan.p_of, plan.t_of = c_of, p_of, t_of
    plan.sorted_pad = sorted_pad
    # gather chunk boundaries (~equal thirds of J)
    bounds = [round(i * J / NCHUNK) for i in range(NCHUNK + 1)]
    plan.chunks = [(bounds[i], bounds[i + 1]) for i in range(NCHUNK)
                   if bounds[i + 1] > bounds[i]]
    return plan


def host_core_inputs(plan, c, input_pos, input_vel, input_action, rest_len):
    """Per-core input tensors."""
    nb = input_pos.shape[0]
    # state slices [P, KSL*M] fp32, layout (t, m) per partition
    sel = plan.c_of == c
    n = np.nonzero(sel)[0]
    real = n < plan.nv
    nr = n[real]
    pos_s = np.zeros((P, KSL, M), np.float32)
    vel_s = np.zeros((P, KSL, M), np.float32)
    pr = input_pos[:, nr].transpose(1, 0, 2).reshape(len(nr), M)
    vr = input_vel[:, nr].transpose(1, 0, 2).reshape(len(nr), M)
    pos_s[plan.p_of[nr], plan.t_of[nr]] = pr
    vel_s[plan.p_of[nr], plan.t_of[nr]] = vr

    # kr [P, J, B] fp32
    e = plan.eslot[c]
    pad = e < 0
    ec = np.clip(e, 0, plan.ne - 1)
    kr = (K_SPRING * rest_len[ec][None]
          * (1.0 + ACT_SCALE * np.tanh(input_action[:, ec]))).astype(np.float32)
    kr[:, pad] = 0.0                                # [B, P, J]
    kr = np.ascontiguousarray(kr.transpose(1, 2, 0).reshape(P, plan.J * nb))

    return {
        "pos0": np.ascontiguousarray(pos_s.reshape(P, KSL * M)),
        "vel0": np.ascontiguousarray(vel_s.reshape(P, KSL * M)),
        "pidx": np.ascontiguousarray(plan.pidx[c]),
        "kr": kr,
    }


def host_table0(plan, input_pos):
    """Initial gather table: fp16 records [NVTOT, M]."""
    tab = np.zeros((NVTOT, M), np.float16)
    n = plan.sorted_pad[plan.sorted_pad < plan.nv]
    rows = (plan.c_of[n] * (KSL * P) + plan.t_of[n] * P + plan.p_of[n])
    tab[rows] = (input_pos[:, n].transpose(1, 0, 2).reshape(len(n), M)
                 ).astype(np.float16)
    return tab


def unpermute_output(plan, trajs, nb):
    """trajs: list of 8 per-core arrays [S+1, P, KSL, M] -> [nb, S+1, NV, 3]."""
    full = np.stack(trajs)                           # [C, S+1, P, KSL, M]
    n = np.arange(plan.nv)
    g = full[plan.c_of[n], :, plan.p_of[n], plan.t_of[n]]   # [NV, S+1, M]
    return np.ascontiguousarray(
        g.reshape(plan.nv, SUBSTEPS + 1, nb, 3).transpose(2, 1, 0, 3))


# ---------------------------------------------------------------------------
# Device kernel
# ---------------------------------------------------------------------------
def _bcast(ap, pos_idx, count):
    dims = [list(x) for x in ap.ap]
    dims.insert(pos_idx, [0, count])
    return bass.AP(ap.tensor, ap.offset, dims)


def build_bass(plan, substeps, nb):
    J = plan.J
    f32 = mybir.dt.float32
    f16 = mybir.dt.float16
    seg = plan.seg

    nc = bass.Bass(num_devices=NCORE)
    pos0 = nc.dram_tensor("pos0", [P, NPM], f32, kind="ExternalInput")
    vel0 = nc.dram_tensor("vel0", [P, NPM], f32, kind="ExternalInput")
    tab0 = nc.dram_tensor("tab0", [NVTOT, M], f16, kind="ExternalInput")
    pidx = nc.dram_tensor("pidx", [P, J], mybir.dt.int32, kind="ExternalInput")
    kr_in = nc.dram_tensor("kr", [P, J * nb], f32, kind="ExternalInput")

    opos = nc.dram_tensor("opos", [substeps + 1, P, NPM], f32,
                          kind="ExternalOutput")
    ovel = nc.dram_tensor("ovel", [substeps + 1, P, NPM], f32,
                          kind="ExternalOutput")

    tab = nc.dram_tensor("tab", [NVTOT, M], f16, kind="Internal")
    cc_in = nc.dram_tensor("cc_in", [KSL * P, M], f16, kind="Internal")

    with _TileContext(nc) as tc:
        with tc.tile_pool(name="state", bufs=1) as pool:
            pos = pool.tile([P, NPM], f32, name="pos")
            vel = pool.tile([P, NPM], f32, name="vel")
            fsum = pool.tile([P, NPM], f32, name="fsum")
            pf16 = pool.tile([P, NPM], f16, name="pf16")    # -pos, fp16
            rem = pool.tile([P, J * M], f16, name="rem")
            rem2 = pool.tile([P, J * M], f16, name="rem2")
            s2f = pool.tile([P, J * nb], f32, name="s2f")
            invt = pool.tile([P, J * nb], f32, name="invt")
            kr_sb = pool.tile([P, J * nb], f32, name="kr_sb")
            pidx_sb = pool.tile([P, J], mybir.dt.int32, name="pidx_sb")
            eps_t = pool.tile([P, 1], f32, name="eps_t")

            rem_v = rem[:].rearrange("p (j m) -> p j m", m=M)
            rem_jbc = rem[:].rearrange("p (j b c) -> p j b c", b=nb, c=3)
            invt_jb = invt[:].rearrange("p (j b) -> p j b", b=nb)
            pf_tm = pf16[:].rearrange("p (t m) -> p t m", m=M)
            fs_tm = fsum[:].rearrange("p (t m) -> p t m", m=M)

            # ---- one-time setup ----
            nc.vector.memset(eps_t[:], float(EPS))
            nc.vector.memset(fsum[:], 0.0)
            nc.sync.dma_start(pos[:], pos0[:])
            nc.sync.dma_start(vel[:], vel0[:])
            nc.sync.dma_start(pidx_sb[:], pidx[:])
            nc.sync.dma_start(kr_sb[:], kr_in[:])
            nc.sync.dma_start(opos[0], pos[:])
            nc.sync.dma_start(ovel[0], vel[:])
            # pf16 = pos in fp16 (matches the table rounding)
            nc.scalar.activation(pf16[:], pos[:],
                                 mybir.ActivationFunctionType.Copy)

            for s in range(substeps):
                TAB = tab0 if s == 0 else tab
                # 1) gather partner records (one column per instruction;
                #    multi-offset indirect DMA is not HW-supported)
                for j in range(J):
                    nc.gpsimd.indirect_dma_start(
                        out=rem_v[:, j:j + 1, :],
                        out_offset=None,
                        in_=TAB[:],
                        in_offset=bass.IndirectOffsetOnAxis(
                            ap=pidx_sb[:, j:j + 1], axis=0),
                    )
                # 2) d = partner - own (per degree class, owner broadcast)
                for (ta, tb, d) in plan.classes:
                    dst = rem_v[:, seg[ta]:seg[tb], :].rearrange(
                        "p (n dd) m -> p n dd m", dd=d)
                    src = _bcast(pf_tm[:, ta:tb, :], 2, d)
                    nc.vector.tensor_tensor(out=dst, in0=dst, in1=src,
                                            op=mybir.AluOpType.subtract)
                # 3) d^2 (ACT) and s2 (DVE)
                for (lo, hi) in plan.chunks:
                    nc.scalar.activation(
                        rem2[:, lo * M:hi * M], rem[:, lo * M:hi * M],
                        mybir.ActivationFunctionType.Square)
                    nc.vector.tensor_reduce(
                        out=s2f[:, lo * nb:hi * nb].rearrange(
                            "p (x one) -> p x one", one=1),
                        in_=rem2[:, lo * M:hi * M].rearrange(
                            "p (x c) -> p x c", c=3),
                        axis=mybir.AxisListType.X, op=mybir.AluOpType.add)
                # 4) len = sqrt(s2+eps); invl = 1/len; t = kr*invl
                nc.scalar.activation(s2f[:], s2f[:],
                                     mybir.ActivationFunctionType.Sqrt,
                                     bias=eps_t[:])
                nc.vector.reciprocal(invt[:], s2f[:])
                nc.vector.tensor_tensor(out=invt[:], in0=kr_sb[:],
                                        in1=invt[:], op=mybir.AluOpType.mult)
                # 5) f = (t - K) * rem   (= true force, since rem = -d)
                nc.vector.scalar_tensor_tensor(
                    out=rem_jbc, in0=_bcast(invt_jb, 3, 3),
                    scalar=float(-K_SPRING), in1=rem_jbc,
                    op0=mybir.AluOpType.add, op1=mybir.AluOpType.mult)
                # 6) segmented reduce -> fsum
                for (ta, tb, d) in plan.classes:
                    src = rem_v[:, seg[ta]:seg[tb], :].rearrange(
                        "p (n dd) m -> p n m dd", dd=d)
                    nc.vector.tensor_reduce(
                        out=fs_tm[:, ta:tb, :], in_=src,
                        axis=mybir.AxisListType.X, op=mybir.AluOpType.add)
                # 7) integrate (fp32): vel = (vel + DT*f + DT*G_y)*DAMP;
                #    pos += DT*vel
                # fsum holds (t-K)*d = -f_true, so integrate with -DT
                nc.vector.scalar_tensor_tensor(
                    out=vel[:], in0=fsum[:], scalar=float(-DT / MASS),
                    in1=vel[:], op0=mybir.AluOpType.mult,
                    op1=mybir.AluOpType.add)
                yv = vel[:].rearrange("p (t b c) -> p t b c",
                                      b=nb, c=3)[:, :, :, 1:2]
                nc.vector.tensor_scalar_add(yv, yv, float(GRAVITY_Y * DT))
                nc.scalar.activation(vel[:], vel[:],
                                     mybir.ActivationFunctionType.Copy,
                                     scale=float(DAMP))
                nc.vector.scalar_tensor_tensor(
                    out=pos[:], in0=vel[:], scalar=float(DT),
                    in1=pos[:], op0=mybir.AluOpType.mult,
                    op1=mybir.AluOpType.add)
                # 8) outputs + next table
                nc.sync.dma_start(opos[s + 1], pos[:])
                nc.sync.dma_start(ovel[s + 1], vel[:])
                if s < substeps - 1:
                    nc.scalar.activation(pf16[:], pos[:],
                                         mybir.ActivationFunctionType.Copy)
                    nc.sync.dma_start(
                        cc_in[:].rearrange("(t p) m -> p t m", p=P),
                        pf_tm)
                    nc.gpsimd.collective_compute(
                        "AllGather", mybir.AluOpType.bypass,
                        replica_groups=[list(range(NCORE))],
                        ins=[cc_in[:]], outs=[tab[:]],
                    )

    return nc


# ---------------------------------------------------------------------------
# Entry point
# ---------------------------------------------------------------------------
_cache = {}


def _get_plan_and_bass(edges, nv, ne, substeps, nb):
    kh = (hash(edges.tobytes()), nv, ne, substeps, nb)
    if kh not in _cache:
        plan = build_plan(edges, nv, ne)
        nc = build_bass(plan, substeps, nb)
        _cache[kh] = (plan, nc)
    return _cache[kh]


def kernel(input_action, input_pos, input_vel, rest_len, edges):
    input_action = np.asarray(input_action, np.float32)
    input_pos = np.asarray(input_pos, np.float32)
    input_vel = np.asarray(input_vel, np.float32)
    rest_len = np.asarray(rest_len, np.float32)
    edges = np.asarray(edges, np.int32)

    nb, nv, _ = input_pos.shape
    ne = edges.shape[0]
    plan, nc = _get_plan_and_bass(edges, nv, ne, SUBSTEPS, nb)

    tab0 = host_table0(plan, input_pos)
    in_maps = []
    for c in range(NCORE):
        im = host_core_inputs(plan, c, input_pos, input_vel,
                              input_action, rest_len)
        im["tab0"] = tab0
        in_maps.append(im)
    res = run_bass_kernel_spmd(nc, in_maps, core_ids=list(range(NCORE)))

    tp = [res.results[c]["opos"].reshape(SUBSTEPS + 1, P, KSL, M)
          for c in range(NCORE)]
    tv = [res.results[c]["ovel"].reshape(SUBSTEPS + 1, P, KSL, M)
          for c in range(NCORE)]
    out_pos = unpermute_output(plan, tp, nb)
    out_vel = unpermute_output(plan, tv, nb)
    return out_pos, out_vel
